# revision 47
# baseline (speedup 1.0000x reference)
"""Trainium2 Bass kernel for nn_MultiHeadAttention_66391604462494.

Strategy (tensor-parallel over heads, 8 cores x 2 heads). The kernel is
HBM-DMA-bound at fp16 (fp8 anywhere on the value path measures 3-7% rel err
-- softmax attention is a weighted mean of random-sign values, so elementwise
quantization noise does NOT average down -- so all tensors stay 16-bit).
The optimization over the original version attacks the dominant DMA stream,
the per-(head,batch) comb = exp(bias)*mask tensor (67 MB/core):

  - Loops run tqc-outer so each head's exp(bias[h]) tile streams ONCE
    (16.8 MB/core total) and is reused across all batches.
  - For the first KDEV=2 batches the mask ships as 1-bit packs (0.25 MB each)
    which gpsimd expands to {0x0000,0xFFFF} int16 lane masks
    ((bits << (15-j)) >>arith 15); DVE applies exp(bias) with a fp16 2x
    tensor_tensor mult, then the mask with a bitwise-AND tensor_tensor
    (fp16 AND 0xFFFF = identity, AND 0x0000 = +0.0, which drops out of the
    PV matmul and the rowsum). Remaining batches keep the host comb path.
    Net: comb DMA 67 -> 50.4 MB/core, DMA busy ~378 -> ~334 us/core; the
    bit-expansion runs as 16 batched DVE shift passes per tqc covering all
    KDEV batches at once (the DVE ALU is 32-bit internally, so the sign
    trick is << (31-j) >> 31 with int16 truncation).
  - PSUM evacuations are split across engines to balance: outproj + PV
    unloads on ACT (scalar.copy), projections on DVE; softmax row-sums come
    free as a ones-column in the PV matmul; normalization happens per
    (b,h,tqc) straight out of PSUM.
  - Device per core otherwise as before: QKV projections (PE, K=1024),
    PE-transposed v blocks, scores^T = k^T.T @ q^T per head (K=64),
    exp on ACT, PV accumulation over tk, partial^T = Wo_c^T.T @ attn.
  - Host: pre-transpose/tile inputs fp16, build eb/bitmask/comb tensors,
    sum the 8 per-core partials, transpose back, add bo.
"""

import os
import sys

import numpy as np

for _p in ("/opt/trn_rl_repo", "/root/.axon_site/_ro/trn_rl_repo"):
    if os.path.isdir(_p) and _p not in sys.path:
        sys.path.insert(0, _p)

import concourse.bass as bass  # noqa: E402
import concourse.mybir as mybir  # noqa: E402
import concourse.tile as tile  # noqa: E402
from concourse import bacc  # noqa: E402
from concourse.bass import ds  # noqa: E402
from concourse.bass_utils import run_bass_kernel_spmd  # noqa: E402
from concourse.masks import make_identity  # noqa: E402

B, S, D, H = 4, 2048, 1024, 16
DK = D // H          # 64
T = B * S            # 8192
NCORES = 8
HPC = H // NCORES    # 2 heads per core
JC = HPC * DK        # 128 = per-core slice of the head dim
NTQ = S // 512       # 4 tq chunks per batch
NTK = S // 128       # 16 tk tiles per batch
NDT = D // 128       # 8 D tiles
F16 = mybir.dt.float16
F32 = mybir.dt.float32
F8 = mybir.dt.float8e4
EXP = mybir.ActivationFunctionType.Exp
MULT = mybir.AluOpType.mult
# Host scales Wq/Wk/Wv by 16 so fp8e4 weight entries (~N(0, 1/32^2)) sit in
# the normal range; q,k each carry 16x -> scores carry 256x, plus the usual
# 1/sqrt(DK)=1/8 -> fold 1/(256*8) into the exp's free scale immediate.
WSCALE = 1.0
EXP_SCALE = 1.0


DEBUG_DUMPS = False
TIMING_REPS = 0  # when >0, wrap the body in a For_i repeat loop (bench only)
USE_DR = False   # DoubleRow fp8 projection matmuls
KDEV = 2         # batches whose mask*ebias is combined on-device (0 => all host comb)
I16 = mybir.dt.int16
AND = mybir.AluOpType.bitwise_and
SHL = mybir.AluOpType.logical_shift_left
SAR = mybir.AluOpType.arith_shift_right


def _emit(nc, tc, qt, kt, vt, wq, wk, wv, wo, cb, out, dbg=None):
    with (
        tc.tile_pool(name="wpool", bufs=1) as wpool,
        tc.tile_pool(name="inpool", bufs=2) as inpool,
        tc.tile_pool(name="qkv", bufs=2) as qkv,
        tc.tile_pool(name="probs", bufs=2) as probsp,
        tc.tile_pool(name="comb", bufs=2) as combp,
        tc.tile_pool(name="norm", bufs=1) as normp,
        tc.tile_pool(name="norm2", bufs=2) as normp2,
        tc.tile_pool(name="attn", bufs=2) as attnp,
        tc.tile_pool(name="outp", bufs=2) as outp,
        tc.tile_pool(name="pp2", bufs=2, space="PSUM") as pp2,
        tc.tile_pool(name="pp1", bufs=3, space="PSUM") as pp1,
        tc.tile_pool(name="ppt", bufs=1, space="PSUM") as ppt,
    ):
        # ---- constants / weights (one-time) ----
        # QKV proj weights in fp8 DoubleRow layout: [p, dp, two, j] packs k-tile
        # pairs (dt = 2*dp + two) so each matmul contracts 256 rows at 2x rate
        wq_sb = wpool.tile([128, NDT // 2, 2, JC], F16, name="wq_sb")
        wk_sb = wpool.tile([128, NDT // 2, 2, JC], F16, name="wk_sb")
        wv_sb = wpool.tile([128, NDT // 2, 2, JC], F16, name="wv_sb")
        wo_sb = wpool.tile([128, NDT, 128], F16, name="wo_sb")
        nc.sync.dma_start(
            wq_sb[:], wq.ap().rearrange("(dp two p) j -> p dp two j", p=128, two=2)
        )
        nc.sync.dma_start(
            wk_sb[:], wk.ap().rearrange("(dp two p) j -> p dp two j", p=128, two=2)
        )
        nc.sync.dma_start(
            wv_sb[:], wv.ap().rearrange("(dp two p) j -> p dp two j", p=128, two=2)
        )
        nc.sync.dma_start(wo_sb[:], wo.ap().rearrange("p (dt o) -> p dt o", dt=NDT))
        ident = wpool.tile([128, 128], F16, name="ident")
        make_identity(nc, ident[:])

        qt_r = qt.ap()
        kt_r = kt.ap()
        vt_r = vt.ap()

        import contextlib
        loop_ctx = (
            tc.For_i(0, TIMING_REPS, 1) if TIMING_REPS > 0 else contextlib.nullcontext()
        )
        with loop_ctx:
          for b in range(B):
            # ---- projections for batch b: q^T, k^T [128j, 2048t] fp16 ----
            qT = qkv.tile([128, S], F16, tag="qT", name=f"qT_{b}")
            kT = qkv.tile([128, S], F16, tag="kT", name=f"kT_{b}")
            vT = qkv.tile([128, S], F16, tag="vT", name=f"vT_{b}")
            for src_r, wsb, dst in ((qt_r, wq_sb, qT), (kt_r, wk_sb, kT), (vt_r, wv_sb, vT)):
                for tci in range(NTQ):
                    xin = inpool.tile([128, NDT // 2, 2, 512], F16, tag="xin", name=f"xin_{b}_{tci}")
                    nc.sync.dma_start(
                        xin[:],
                        src_r[b * NTQ + tci].rearrange("p (dp two) t -> p dp two t", two=2),
                    )
                    ps = pp1.tile([128, 512], F32, tag="mm", name=f"proj_{b}_{tci}")
                    if USE_DR:
                        for dpi in range(NDT // 2):
                            nc.tensor.matmul(
                                ps[:], lhsT=wsb[:, dpi], rhs=xin[:, dpi],
                                start=(dpi == 0), stop=(dpi == NDT // 2 - 1),
                                perf_mode=mybir.MatmulPerfMode.DoubleRow,
                            )
                    else:
                        for dti in range(NDT):
                            nc.tensor.matmul(
                                ps[:],
                                lhsT=wsb[:, dti // 2, dti % 2, :],
                                rhs=xin[:, dti // 2, dti % 2, :],
                                start=(dti == 0), stop=(dti == NDT - 1),
                            )
                    nc.vector.tensor_copy(dst[:, ds(tci * 512, 512)], ps[:])

            # ---- v^T -> v[t, dk] blocks (+ ones column at dk=64) ----
            v0 = qkv.tile([128, NTK, 65], F16, tag="v0", name=f"v0_{b}")
            v1 = qkv.tile([128, NTK, 65], F16, tag="v1", name=f"v1_{b}")
            # column 64 of v' is all-ones: the PV matmul then yields the probs
            # row-sum on PSUM partition 64 for free
            nc.gpsimd.memset(v0[:, :, 64:65], 1.0)
            nc.gpsimd.memset(v1[:, :, 64:65], 1.0)
            for blk in range(NTK):
                pst = ppt.tile([128, 128], F16, tag="vtr", name=f"vtr_{b}_{blk}")
                nc.tensor.transpose(pst[:], vT[:, ds(blk * 128, 128)], ident[:])
                nc.vector.tensor_copy(v0[:, blk, 0:64], pst[:, 0:64])
                nc.vector.tensor_copy(v1[:, blk, 0:64], pst[:, 64:128])

            if dbg is not None and b == 0:
                nc.sync.dma_start(dbg["qT0"].ap(), qT[:])
                nc.sync.dma_start(dbg["kT0"].ap(), kT[:])
                nc.sync.dma_start(dbg["v00"].ap(), v0[:])
                nc.sync.dma_start(dbg["v10"].ap(), v1[:])

            # ---- attention for batch b ----
            unorm = [None, None]
            for h in range(HPC):
                unorm[h] = normp.tile([65, NTQ, 512], F32, tag=f"unorm{h}", name=f"unorm_{b}_{h}")
            for tqc in range(NTQ):
                probs = [None, None]
                comb = [None, None]
                for h in range(HPC):
                    probs[h] = probsp.tile([128, NTK, 512], F16, tag=f"probs{h}", name=f"pr_{b}_{tqc}_{h}", bufs=1)
                    comb[h] = combp.tile([128, NTK, 512], F16, tag=f"comb{h}", name=f"cb_{b}_{tqc}_{h}", bufs=1)
                    nc.sync.dma_start(comb[h][:], cb.ap()[h, b, tqc])
                # interleave the two heads' K=64 matmuls: adjacent MMs target
                # disjoint PE row groups (base partitions 0 / 64) and run
                # concurrently in the array
                for tkp in range(NTK // 2):
                    for h in range(HPC):
                        ps2 = pp2.tile([128, 1024], F32, tag="s2", name=f"sc_{b}_{tqc}_{h}_{tkp}")
                        for half in range(2):
                            tk = tkp * 2 + half
                            nc.tensor.matmul(
                                ps2[:, ds(half * 512, 512)],
                                lhsT=kT[ds(h * 64, 64), ds(tk * 128, 128)],
                                rhs=qT[ds(h * 64, 64), ds(tqc * 512, 512)],
                                start=True, stop=True,
                            )
                        nc.scalar.activation(
                            probs[h][:, ds(tkp * 2, 2), :], ps2[:], EXP, scale=EXP_SCALE
                        )
                for h in range(HPC):
                    vh = v0 if h == 0 else v1
                    nc.vector.tensor_tensor(probs[h][:], probs[h][:], comb[h][:], op=MULT)
                    pv = pp1.tile([128, 512], F32, tag="mm", name=f"pv_{b}_{tqc}_{h}")
                    for tk in range(NTK):
                        nc.tensor.matmul(
                            pv[0:65, :], lhsT=vh[:, tk, :], rhs=probs[h][:, tk, :],
                            start=(tk == 0), stop=(tk == NTK - 1),
                        )
                    # scalar engine takes this PSUM evacuation to offload DVE
                    nc.scalar.copy(unorm[h][:, tqc, :], pv[0:65, :])
                    if dbg is not None and b == 0 and tqc == 0:
                        nc.sync.dma_start(dbg[f"probs0_{h}"].ap(), probs[h][:])

            # ---- normalize + output projection for batch b ----
            attn16 = attnp.tile([128, S], F16, tag="attn16", name=f"attn_{b}")
            for h in range(HPC):
                # rowsum lives on partition 64; partition_broadcast reads
                # physical partition 0 regardless of AP offset, so hop it
                # through a partition-0 tile first
                rs0 = normp2.tile([1, S], F32, tag="rs0", name=f"rs0_{b}_{h}")
                nc.vector.tensor_copy(
                    rs0[:], unorm[h][64:65, :, :].rearrange("p a b -> p (a b)")
                )
                rsb = normp2.tile([64, S], F32, tag="rsb", name=f"rsb_{b}_{h}")
                nc.gpsimd.partition_broadcast(rsb[:], rs0[:])
                bcast = normp.tile([64, S], F32, tag="bcast", name=f"bc_{b}_{h}")
                nc.vector.reciprocal_approx_fast(bcast[:], rsb[:])
                nc.vector.tensor_tensor(
                    attn16[ds(h * 64, 64), :],
                    unorm[h][0:64, :, :].rearrange("p a b -> p (a b)"),
                    bcast[:],
                    op=MULT,
                )
                if dbg is not None and b == 0:
                    nc.sync.dma_start(dbg[f"unorm0_{h}"].ap(), unorm[h][:].rearrange("p a b -> p (a b)"))
                    nc.sync.dma_start(dbg[f"recip0_{h}"].ap(), rsb[:])
                    nc.sync.dma_start(dbg[f"bcast0_{h}"].ap(), bcast[:])
            if dbg is not None and b == 0:
                nc.sync.dma_start(dbg["attn0"].ap(), attn16[:])
            for tqc in range(NTQ):
                for dp in range(NDT // 2):
                    po = pp2.tile([128, 1024], F32, tag="s2", name=f"op_{b}_{tqc}_{dp}")
                    for half in range(2):
                        nc.tensor.matmul(
                            po[:, ds(half * 512, 512)],
                            lhsT=wo_sb[:, dp * 2 + half, :],
                            rhs=attn16[:, ds(tqc * 512, 512)],
                            start=True, stop=True,
                        )
                    ost = outp.tile([128, 1024], F16, tag="ost", name=f"ost_{b}_{tqc}_{dp}")
                    # split PSUM evacuation between ACT and DVE to balance engines
                    if tqc == 0 and dp < 2:
                        nc.scalar.copy(ost[:], po[:])
                    else:
                        nc.vector.tensor_copy(ost[:], po[:])
                    nc.sync.dma_start(out.ap()[b, tqc, dp], ost[:])


def _emit2(nc, tc, qt, kt, vt, wq, wk, wv, wo, cb, eb, mb, out):
    """tqc-outer variant: exp(bias) streamed once (16.8 MB) and reused across
    batches; masks for the first KDEV batches ship as 1-bit packs expanded to
    {0,0xFFFF} fp16 patterns on gpsimd, applied to probs via a bitwise-AND
    tensor_tensor; remaining batches keep the host comb path."""
    with (
        tc.tile_pool(name="wpool", bufs=1) as wpool,
        tc.tile_pool(name="inpool", bufs=2) as inpool,
        tc.tile_pool(name="qkvp", bufs=1) as qkvp,
        tc.tile_pool(name="vtp", bufs=1) as vtp,
        tc.tile_pool(name="probs", bufs=2) as probsp,
        tc.tile_pool(name="cmul", bufs=1) as cmulp,
        tc.tile_pool(name="maskp", bufs=1) as maskp,
        tc.tile_pool(name="bitsp", bufs=1) as bitsp,
        tc.tile_pool(name="norm2", bufs=1) as normp2,
        tc.tile_pool(name="attn", bufs=1) as attnp,
        tc.tile_pool(name="outp", bufs=2) as outp,
        tc.tile_pool(name="pp2", bufs=2, space="PSUM") as pp2,
        tc.tile_pool(name="pp1", bufs=3, space="PSUM") as pp1,
        tc.tile_pool(name="ppt", bufs=1, space="PSUM") as ppt,
    ):
        wq_sb = wpool.tile([128, NDT, JC], F16, name="wq_sb")
        wk_sb = wpool.tile([128, NDT, JC], F16, name="wk_sb")
        wv_sb = wpool.tile([128, NDT, JC], F16, name="wv_sb")
        wo_sb = wpool.tile([128, NDT, 128], F16, name="wo_sb")
        nc.sync.dma_start(wq_sb[:], wq.ap().rearrange("(dt p) j -> p dt j", p=128))
        nc.sync.dma_start(wk_sb[:], wk.ap().rearrange("(dt p) j -> p dt j", p=128))
        nc.sync.dma_start(wv_sb[:], wv.ap().rearrange("(dt p) j -> p dt j", p=128))
        nc.sync.dma_start(wo_sb[:], wo.ap().rearrange("p (dt o) -> p dt o", dt=NDT))
        ident = wpool.tile([128, 128], F16, name="ident")
        make_identity(nc, ident[:])
        c15 = wpool.tile([128, max(KDEV, 1), NTK, 32], I16, name="c15")
        nc.gpsimd.memset(c15[:], 31)
        # per-partition shift amounts 15-j (bitvec STT immediates must be
        # integer-typed; bass emits fp32 imms, so use [128,1] i16 APs instead)
        shamt = wpool.tile([128, 16], I16, name="shamt")
        for j in range(16):
            # DVE ALU is 32-bit internally: put mask bit j at bit 31, then
            # arithmetic-shift down so the int16 truncation is 0xFFFF/0x0000
            nc.gpsimd.memset(shamt[:, j:j + 1], 31 - j)

        qt_r = qt.ap()
        kt_r = kt.ap()
        vt_r = vt.ap()

        import contextlib
        loop_ctx = (
            tc.For_i(0, TIMING_REPS, 1) if TIMING_REPS > 0 else contextlib.nullcontext()
        )
        with loop_ctx:
            # ---- phase A: projections + v-blocks for all batches ----
            qTs, kTs, v0s, v1s = [], [], [], []
            for b in range(B):
                qT = qkvp.tile([128, S], F16, tag=f"qT{b}", name=f"qT_{b}")
                kT = qkvp.tile([128, S], F16, tag=f"kT{b}", name=f"kT_{b}")
                vT = vtp.tile([128, S], F16, tag="vT", name=f"vT_{b}")
                for src_r, wsb, dst in ((qt_r, wq_sb, qT), (kt_r, wk_sb, kT), (vt_r, wv_sb, vT)):
                    for tci in range(NTQ):
                        xin = inpool.tile([128, NDT, 512], F16, tag="xin", name=f"x2_{b}_{tci}")
                        nc.sync.dma_start(xin[:], src_r[b * NTQ + tci])
                        ps = pp1.tile([128, 512], F32, tag="mm", name=f"pj2_{b}_{tci}")
                        for dti in range(NDT):
                            nc.tensor.matmul(
                                ps[:], lhsT=wsb[:, dti, :], rhs=xin[:, dti, :],
                                start=(dti == 0), stop=(dti == NDT - 1),
                            )
                        nc.scalar.copy(dst[:, ds(tci * 512, 512)], ps[:])
                v0 = qkvp.tile([128, NTK, 65], F16, tag=f"v0{b}", name=f"v0_{b}")
                v1 = qkvp.tile([128, NTK, 65], F16, tag=f"v1{b}", name=f"v1_{b}")
                nc.gpsimd.memset(v0[:, :, 64:65], 1.0)
                nc.gpsimd.memset(v1[:, :, 64:65], 1.0)
                for blk in range(NTK):
                    pst = ppt.tile([128, 128], F16, tag="vtr", name=f"vt2_{b}_{blk}")
                    nc.tensor.transpose(pst[:], vT[:, ds(blk * 128, 128)], ident[:])
                    nc.vector.tensor_copy(v0[:, blk, 0:64], pst[:, 0:64])
                    nc.vector.tensor_copy(v1[:, blk, 0:64], pst[:, 64:128])
                qTs.append(qT)
                kTs.append(kT)
                v0s.append(v0)
                v1s.append(v1)

            attn16s = [
                attnp.tile([128, S], F16, tag=f"at{b}", name=f"at_{b}") for b in range(B)
            ]

            # ---- phase B: attention, tqc-outer ----
            m16s = [None] * KDEV
            for tqc in range(NTQ):
                for h in range(HPC):
                    ebt = None
                    if KDEV > 0:
                        ebt = cmulp.tile([128, NTK, 512], F16, tag="eb", name=f"eb_{tqc}_{h}")
                        nc.sync.dma_start(ebt[:], eb.ap()[h, tqc])
                    if KDEV > 0 and h == 0:
                        bt = bitsp.tile([128, KDEV, NTK, 32], I16, tag="bits", name=f"bt_{tqc}")
                        nc.sync.dma_start(bt[:], mb.ap()[tqc])
                        m16all = maskp.tile(
                            [128, KDEV, NTK, 512], I16, tag="m16", name=f"m16_{tqc}"
                        )
                        # expand bit j of each 16-bit word w to tq column
                        # j*32+w as 0x0000/0xFFFF, all KDEV batches per pass
                        for j in range(16):
                            nc.vector.scalar_tensor_tensor(
                                m16all[:, :, :, ds(j * 32, 32)], bt[:],
                                shamt[:, j:j + 1], c15[:], op0=SHL, op1=SAR,
                            )
                        for b in range(KDEV):
                            m16s[b] = m16all[:, b]
                    for b in range(B):
                        dev = b < KDEV
                        probs = probsp.tile(
                            [128, NTK, 512], F16, tag="pr", name=f"p2_{tqc}_{h}_{b}"
                        )
                        qT, kT = qTs[b], kTs[b]
                        # 2 tk-tiles per scores psum group (double-buffered so
                        # QK matmuls overlap the previous group's exp)
                        for quad in range(NTK // 2):
                            ps4 = pp2.tile([128, 2, 512], F32, tag="s2", name=f"s2_{tqc}_{h}_{b}_{quad}")
                            for tkq in range(2):
                                tk = quad * 2 + tkq
                                nc.tensor.matmul(
                                    ps4[:, tkq, :],
                                    lhsT=kT[ds(h * 64, 64), ds(tk * 128, 128)],
                                    rhs=qT[ds(h * 64, 64), ds(tqc * 512, 512)],
                                    start=True, stop=True,
                                )
                            nc.scalar.activation(
                                probs[:, ds(quad * 2, 2), :], ps4[:], EXP, scale=EXP_SCALE
                            )
                        if dev:
                            nc.vector.tensor_tensor(probs[:], probs[:], ebt[:], op=MULT)
                            nc.vector.tensor_tensor(
                                probs[:].bitcast(I16), probs[:].bitcast(I16),
                                m16s[b], op=AND,
                            )
                        else:
                            combt = cmulp.tile(
                                [128, NTK, 512], F16, tag="cb", name=f"c2_{tqc}_{h}_{b}"
                            )
                            nc.sync.dma_start(combt[:], cb.ap()[h, b - KDEV, tqc])
                            nc.vector.tensor_tensor(probs[:], probs[:], combt[:], op=MULT)
                        vh = v0s[b] if h == 0 else v1s[b]
                        pv = pp1.tile([128, 512], F32, tag="mm", name=f"pv2_{tqc}_{h}_{b}")
                        for tk in range(NTK):
                            nc.tensor.matmul(
                                pv[0:65, :], lhsT=vh[:, tk, :], rhs=probs[:, tk, :],
                                start=(tk == 0), stop=(tk == NTK - 1),
                            )
                        # normalize in place: rowsum sits on psum partition 64;
                        # hop to partition 0, then reciprocal + broadcast
                        # (partition_broadcast reads phys partition 0)
                        rs0 = normp2.tile([1, 512], F32, tag="rs0", name=f"r2_{tqc}_{h}_{b}")
                        if h == 0:
                            nc.vector.tensor_copy(rs0[:], pv[64:65, :])
                        else:
                            nc.scalar.copy(rs0[:], pv[64:65, :])
                        rc0 = normp2.tile([1, 512], F32, tag="rc0", name=f"rc_{tqc}_{h}_{b}")
                        nc.vector.reciprocal_approx_fast(rc0[:], rs0[:])
                        bcast = normp2.tile([64, 512], F32, tag="bc", name=f"bc_{tqc}_{h}_{b}")
                        nc.gpsimd.partition_broadcast(bcast[:], rc0[:])
                        nc.vector.tensor_tensor(
                            attn16s[b][ds(h * 64, 64), ds(tqc * 512, 512)],
                            pv[0:64, :], bcast[:], op=MULT,
                        )
                        # outproj for batch b once its attn16 is complete
                        # (last tqc, last head) -- earlier batches' outproj
                        # overlaps the remaining batches' attention work
                        if h == HPC - 1 and tqc == NTQ - 1:
                            for tqc2 in range(NTQ):
                                for dp in range(NDT // 2):
                                    po = pp2.tile([128, 1024], F32, tag="s2", name=f"o2_{tqc2}_{b}_{dp}")
                                    for half in range(2):
                                        nc.tensor.matmul(
                                            po[:, ds(half * 512, 512)],
                                            lhsT=wo_sb[:, dp * 2 + half, :],
                                            rhs=attn16s[b][:, ds(tqc2 * 512, 512)],
                                            start=True, stop=True,
                                        )
                                    ost = outp.tile([128, 1024], F16, tag="ost", name=f"os2_{b}_{tqc2}_{dp}")
                                    if tqc2 == 3 and dp < 2:
                                        nc.vector.tensor_copy(ost[:], po[:])
                                    else:
                                        nc.scalar.copy(ost[:], po[:])
                                    nc.sync.dma_start(out.ap()[b, tqc2, dp], ost[:])



_NC_CACHE = None


def _build_bass():
    global _NC_CACHE
    if _NC_CACHE is not None:
        return _NC_CACHE
    nc = bacc.Bacc("TRN2", target_bir_lowering=False, debug=False, num_devices=NCORES)
    # pre-tiled on host: [b*tci, p, dt, t] so every DMA is one contiguous 1 MB read
    qt = nc.dram_tensor("qt", [B * NTQ, 128, NDT, 512], F16, kind="ExternalInput")
    kt = nc.dram_tensor("kt", [B * NTQ, 128, NDT, 512], F16, kind="ExternalInput")
    vt = nc.dram_tensor("vt", [B * NTQ, 128, NDT, 512], F16, kind="ExternalInput")
    wq = nc.dram_tensor("wq", [D, JC], F16, kind="ExternalInput")
    wk = nc.dram_tensor("wk", [D, JC], F16, kind="ExternalInput")
    wv = nc.dram_tensor("wv", [D, JC], F16, kind="ExternalInput")
    wo = nc.dram_tensor("wo", [JC, D], F16, kind="ExternalInput")
    # pre-tiled on host: [h, b, tqc, tki, tko, tq] — contiguous 2 MB per DMA
    cb = nc.dram_tensor(
        "cb", [HPC, max(B - KDEV, 1), NTQ, 128, NTK, 512], F16, kind="ExternalInput"
    )
    eb = mb = None
    if KDEV > 0:
        eb = nc.dram_tensor("eb", [HPC, NTQ, 128, NTK, 512], F16, kind="ExternalInput")
        mb = nc.dram_tensor("mb", [NTQ, 128, KDEV, NTK, 32], I16, kind="ExternalInput")
    out = nc.dram_tensor("out", [B, NTQ, NDT // 2, 128, 1024], F16, kind="ExternalOutput")
    dbg = None
    if DEBUG_DUMPS:
        dbg = {
            "qT0": nc.dram_tensor("qT0", [128, S], F16, kind="ExternalOutput"),
            "kT0": nc.dram_tensor("kT0", [128, S], F16, kind="ExternalOutput"),
            "v00": nc.dram_tensor("v00", [128, NTK, 65], F16, kind="ExternalOutput"),
            "v10": nc.dram_tensor("v10", [128, NTK, 65], F16, kind="ExternalOutput"),
            "probs0_0": nc.dram_tensor("probs0_0", [128, NTK, 512], F16, kind="ExternalOutput"),
            "probs0_1": nc.dram_tensor("probs0_1", [128, NTK, 512], F16, kind="ExternalOutput"),
            "unorm0_0": nc.dram_tensor("unorm0_0", [65, S], F32, kind="ExternalOutput"),
            "unorm0_1": nc.dram_tensor("unorm0_1", [65, S], F32, kind="ExternalOutput"),
            "recip0_0": nc.dram_tensor("recip0_0", [64, S], F32, kind="ExternalOutput"),
            "recip0_1": nc.dram_tensor("recip0_1", [64, S], F32, kind="ExternalOutput"),
            "bcast0_0": nc.dram_tensor("bcast0_0", [64, S], F32, kind="ExternalOutput"),
            "bcast0_1": nc.dram_tensor("bcast0_1", [64, S], F32, kind="ExternalOutput"),
            "attn0": nc.dram_tensor("attn0", [128, S], F16, kind="ExternalOutput"),
        }
    with tile.TileContext(nc) as tc:
        _emit2(nc, tc, qt, kt, vt, wq, wk, wv, wo, cb, eb, mb, out)
    nc.finalize()
    _NC_CACHE = nc
    return nc


def _tile_xT(X):
    # [T, D] -> X^T tiled as [b*tci, p, dt, t] (contiguous per [128, NDT, 512] tile)
    xt = X.reshape(T, D).T.astype(np.float16)          # [D, T] = [dt*128+p, ...]
    xt = xt.reshape(NDT, 128, B * NTQ, 512)            # [dt, p, b*tci, t]
    return np.ascontiguousarray(np.transpose(xt, (2, 1, 0, 3)))


def _prepare_in_maps(Q, K, V, mask, attn_bias, Wq, Wk, Wv):
    f16 = np.float16
    qt = _tile_xT(Q)
    kt = _tile_xT(K)
    vt = _tile_xT(V)
    # mask transposed per batch, as bool [B, Sk, Sq]
    mT = (np.transpose(mask[:, 0], (0, 2, 1)) != 0)
    in_maps = []
    for c in range(NCORES):
        sl = slice(c * JC, (c + 1) * JC)
        wq_c = np.ascontiguousarray(Wq[sl].T / np.sqrt(DK)).astype(f16)
        wk_c = np.ascontiguousarray(Wk[sl].T).astype(f16)
        wv_c = np.ascontiguousarray(Wv[sl].T).astype(f16)
        wo_c = np.ascontiguousarray(_WO_GLOBAL[:, sl].T).astype(f16)
        comb = np.empty((HPC, max(B - KDEV, 1), NTQ, 128, NTK, 512), f16)
        eba = np.empty((HPC, NTQ, 128, NTK, 512), f16)
        for hh in range(HPC):
            ebT = np.exp(attn_bias[0, c * HPC + hh].astype(np.float64)).T.astype(f16)
            eb4 = ebT.reshape(NTK, 128, NTQ, 512)      # [tko, tki, tqc, tq]
            eba[hh] = np.transpose(eb4, (2, 1, 0, 3))
            for b in range(KDEV, B):
                cbb = np.where(mT[b], ebT, f16(0))     # [tk, tq]
                cbb = cbb.reshape(NTK, 128, NTQ, 512)  # [tko, tki, tqc, tq]
                comb[hh, b - KDEV] = np.transpose(cbb, (2, 1, 0, 3))
        # 1-bit mask packs for the on-device batches: bit j of word w covers
        # tq column tqc*512 + j*32 + w (see _emit2's expansion)
        mba = np.empty((NTQ, 128, max(KDEV, 1), NTK, 32), np.uint16)
        for b in range(KDEV):
            m5 = mT[b].reshape(NTK, 128, NTQ, 16, 32).astype(np.uint16)
            bits = np.zeros((NTK, 128, NTQ, 32), np.uint16)
            for j in range(16):
                bits |= m5[:, :, :, j, :] << np.uint16(j)
            mba[:, :, b] = np.transpose(bits, (2, 1, 0, 3))
        im = {
            "qt": qt, "kt": kt, "vt": vt,
            "wq": wq_c, "wk": wk_c, "wv": wv_c, "wo": wo_c,
            "cb": comb,
        }
        if KDEV > 0:
            im["eb"] = eba
            im["mb"] = mba.view(np.int16)
        in_maps.append(im)
    return in_maps


_WO_GLOBAL = None


def _postprocess(results, bo):
    acc = np.zeros((D, T), np.float32)
    for r in results:
        arr = r["out"].reshape(B, NTQ, NDT // 2, 128, 2, 512)
        acc += np.transpose(arr, (2, 4, 3, 0, 1, 5)).reshape(D, T)
    out = acc.T + bo[None, :].astype(np.float32)
    return out.reshape(B, S, D).astype(np.float32)


def _run(inputs, trace=False):
    global _WO_GLOBAL
    _WO_GLOBAL = np.asarray(inputs["Wo"], np.float32)
    nc = _build_bass()
    in_maps = _prepare_in_maps(
        np.asarray(inputs["Q"], np.float32), np.asarray(inputs["K"], np.float32),
        np.asarray(inputs["V"], np.float32), np.asarray(inputs["mask"]),
        np.asarray(inputs["attn_bias"], np.float32), np.asarray(inputs["Wq"], np.float32),
        np.asarray(inputs["Wk"], np.float32), np.asarray(inputs["Wv"], np.float32),
    )
    res = run_bass_kernel_spmd(nc, in_maps, core_ids=list(range(NCORES)), trace=trace)
    out = _postprocess(res.results, np.asarray(inputs["bo"], np.float32))
    return out, res


def kernel(**inputs):
    out, _ = _run(inputs, trace=False)
    return out

